# revision 26
# baseline (speedup 1.0000x reference)
"""Trainium2 Bass kernel for nn_Embedded_GCN (gnn_message_passing).

Reference math (B=32, N=4096, C=32, O=64, D=16, K=3):
  A  = softmax(relu(E @ E.T), axis=1)              # [N, N] adaptive adjacency
  T0 = I, T1 = A, T2 = 2A@A - I                    # Chebyshev
  x_g[k]   = T_k @ x_b  for each batch             # [B, K, N, C]
  W[n]     = sum_d E[n,d] * Wp[d]                  # per-node weights [K,C,O]
  out[b,n] = sum_{k,i} x_g[b,n,k,i] W[n,k,i,:] + E[n]@bias_pool

Key restructurings:
  * T2 is never materialized: z1 = A@x, z2 = 2*A@z1 - x (avoids the N^3 matmul).
  * softmax(relu(s)) = max(1, exp(s)) / rowsum  (exp never overflows: logits <~ 50).
  * Scores are computed directly transposed: PT[m, n] = exp-scores with the
    contraction (m) on partitions, so the two propagation hops need no on-chip
    transpose of the 4096x4096 attention matrix.
  * Row sums via ones-matmul on the PE; 1/Z folded into the hop PSUM->SBUF
    epilogue as a per-partition activation scale.
  * Big matmuls run in float32r (FP22 truncation, full PE speed at N>=512).
  * Node-sharding across 8 cores (512 nodes each); z1 is AllGathered (2MB/rank).
  * Per-node grouped GEMM: x_g is permuted to [(k,c), (n,b)] via a DRAM
    round-trip (contiguous-ish chunks both directions), per-node weights are
    generated on the PE into a [97, (n,o)] slab (97th row = bias, matched by a
    ones-row in x_gT), and 512 small [97,32]x[97,64] bf16 matmuls finish it.
"""

import os

import numpy as np
import ml_dtypes

import concourse.bass as bass
import concourse.mybir as mybir
import concourse.tile as tile
from concourse import bacc
from concourse.bass_utils import run_bass_kernel_spmd

F32 = mybir.dt.float32
F32R = mybir.dt.float32r
BF16 = mybir.dt.bfloat16
AF = mybir.ActivationFunctionType

B, N, C, O, D, CHEB_K = 32, 4096, 32, 64, 16, 3
NC_CORES = 8
NL = N // NC_CORES          # 512 nodes per core
BC = B * C                  # 1024
MT = N // 128               # 32 contraction tiles
NT = NL // 128              # 4 local node tiles

LAST_RESULTS = {}


def _register_ntff_hook():
    """Inject antenv.axon_hooks (absent from the container's antenv stub) and
    register the ctypes NTFF-profile hook so trace=True works under axon."""
    import sys
    import types

    try:
        import antenv

        if "antenv.axon_hooks" not in sys.modules:
            mod = types.ModuleType("antenv.axon_hooks")
            mod._hook = None

            def set_axon_ntff_profile_hook(h):
                mod._hook = h

            def get_axon_ntff_profile_hook():
                return mod._hook

            mod.set_axon_ntff_profile_hook = set_axon_ntff_profile_hook
            mod.get_axon_ntff_profile_hook = get_axon_ntff_profile_hook
            sys.modules["antenv.axon_hooks"] = mod
            antenv.axon_hooks = mod

        hooks = sys.modules["antenv.axon_hooks"]
        if hooks.get_axon_ntff_profile_hook() is None:
            from trn_agent_boot.trn_boot import _ntff_profile_via_ctypes

            hook = _ntff_profile_via_ctypes("/opt/axon/libaxon_pjrt.so")
            if hook is not None:
                hooks.set_axon_ntff_profile_hook(hook)
        return True
    except Exception:
        return False


def _build(nc: bacc.Bacc):
    # ---- I/O -------------------------------------------------------------
    et = nc.dram_tensor("et", [D, N], F32, kind="ExternalInput")          # E^T
    et_loc = nc.dram_tensor("et_loc", [D, NL], F32, kind="ExternalInput")
    xt_bf = nc.dram_tensor("xt_bf", [N, BC], BF16, kind="ExternalInput")   # x[b,m,c] -> [m, c*32+b]
    xtl_bf = nc.dram_tensor("xtl_bf", [NL, BC], BF16, kind="ExternalInput")
    xt_loc = nc.dram_tensor("xt_loc", [NL, BC], F32, kind="ExternalInput")
    wp_o = nc.dram_tensor("wp_o", [O, D, CHEB_K * C], BF16, kind="ExternalInput")
    bias_flat = nc.dram_tensor("bias_flat", [1, NL * O], BF16, kind="ExternalInput")
    out_loc = nc.dram_tensor("out_loc", [B, NL, O], F32, kind="ExternalOutput")

    with tile.TileContext(nc) as tc:
        with tc.tile_pool(name="dram", bufs=1, space="DRAM") as dram, \
             tc.tile_pool(name="persist", bufs=1) as persist:

            ag_in_a = dram.tile([NL, 512], BF16, tag="ag_in_a")
            ag_in_b = dram.tile([NL, 512], BF16, tag="ag_in_b")
            ag_out_a = dram.tile([N, 512], BF16, tag="ag_out_a", addr_space="Shared")
            ag_out_b = dram.tile([N, 512], BF16, tag="ag_out_b", addr_space="Shared")
            scr1 = dram.tile([C, NL, B], BF16, tag="scr1")   # z1 as [c, n, b]
            scr2 = dram.tile([C, NL, B], BF16, tag="scr2")   # z2 as [c, n, b]
            r_dram = dram.tile([2, NL], F32, tag="r_dram")

            # ---- small persistent SBUF ------------------------------------
            etl_sb = persist.tile([D, NL], F32R, tag="etl")
            r1 = persist.tile([128, NT], F32, tag="r1")          # 1/Z  per node col nt
            r2 = persist.tile([128, NT], F32, tag="r2")          # 2/Z

            etl_bf = persist.tile([D, NL], BF16, tag="etlbf")
            nc.sync.dma_start(etl_sb[:], et_loc[:, :].bitcast(F32R))
            nc.vector.tensor_copy(etl_bf[:], etl_sb[:].bitcast(F32))

            with tc.tile_pool(name="wtp", bufs=1) as wtp, \
                 tc.tile_pool(name="tstream", bufs=3) as tstream:
                # weight slab, n-major: [ki|bias, (n, o)]
                wt_bf = wtp.tile([97, NL * O], BF16, tag="wt")
                nc.gpsimd.dma_start(wt_bf[96:97, :], bias_flat[:, :])
                # interleaved: free idx = n_hi*512 + o*8 + n_lo, node n = n_hi*8 + n_lo
                wt_i8 = wt_bf[0:96, :].rearrange("p (nh o nl) -> p nh o nl", o=O, nl=8)
                wt_g = wt_bf[:].rearrange("p (nh o nl) -> p nh nl o", o=O, nl=8)

                xgp_cm = tc.tile_pool(name="xg", bufs=1)
                xgp = xgp_cm.__enter__()
                xgs = []
                for ch in range(NT):
                    n0 = ch * 128
                    xg_t = xgp.tile([97, 128 * B], BF16, tag=f"xg{ch}", name=f"xg{ch}")
                    nc.gpsimd.dma_start(
                        xg_t[0:C, :].rearrange("c (n b) -> c n b", b=B),
                        xtl_bf[n0:n0 + 128, :].rearrange("n (c b) -> c n b", b=B),
                    )
                    xgs.append(xg_t)

                wpo_sb = wtp.tile([D, O * CHEB_K * C], BF16, tag="wpo")
                nc.scalar.dma_start(
                    wpo_sb[:].rearrange("d (o k) -> d o k", k=CHEB_K * C),
                    wp_o[:, :, :].transpose((1, 0, 2)),
                )

                with tc.tile_pool(name="ptp", bufs=1) as ptp, \
                     tc.tile_pool(name="stream", bufs=3) as stream:
                    pt = ptp.tile([128, MT * NL], BF16, tag="pt")  # PT[m%128, mt*NL+n]
                    xloc_sb = ptp.tile([128, NT * BC], F32, tag="xloc")

                    # ---- phase B/C: transposed exp-scores + row sums ------
                    ones_f = persist.tile([128, 2], F32, tag="onesf")
                    nc.vector.memset(ones_f[:], 1.0)
                    with tc.tile_pool(name="etp", bufs=1) as etp, \
                         tc.tile_pool(name="ps_sc", bufs=3, space="PSUM") as ps_sc:
                        et_sb = etp.tile([D, N], F32R, tag="et")
                        nc.sync.dma_start(et_sb[:, 0:N // 2], et[:, 0:N // 2].bitcast(F32R))
                        nc.gpsimd.dma_start(et_sb[:, N // 2:], et[:, N // 2:].bitcast(F32R))
                        # 4 interleaved accumulators: short critical path after last exp
                        accs = [etp.tile([128, NL], F32, tag=f"accs{i}", name=f"accs{i}")
                                for i in range(4)]
                        for mt in range(MT):
                            s_ps = ps_sc.tile([128, NL], F32, tag="s")
                            nc.tensor.matmul(
                                s_ps[:],
                                et_sb[:, mt * 128:(mt + 1) * 128],
                                etl_sb[:],
                                start=True, stop=True,
                            )
                            pslice = pt[:, mt * NL:(mt + 1) * NL]
                            nc.scalar.activation(pslice, s_ps[:], AF.Exp)
                            nc.vector.tensor_scalar_max(pslice, pslice, 1.0)
                            a = accs[mt % 4]
                            if mt < 4:
                                nc.vector.tensor_copy(a[:], pslice)
                            else:
                                nc.vector.tensor_tensor(
                                    a[:], a[:], pslice, mybir.AluOpType.add)
                        nc.vector.tensor_tensor(
                            accs[0][:], accs[0][:], accs[1][:], mybir.AluOpType.add)
                        nc.vector.tensor_tensor(
                            accs[2][:], accs[2][:], accs[3][:], mybir.AluOpType.add)
                        nc.vector.tensor_tensor(
                            accs[0][:], accs[0][:], accs[2][:], mybir.AluOpType.add)
                        zs = ps_sc.tile([128, 8], F32, tag="zs")
                        for nt_i in range(NT):
                            nc.tensor.matmul(
                                zs[:, 2 * nt_i:2 * nt_i + 2],
                                accs[0][:, nt_i * 128:(nt_i + 1) * 128],
                                ones_f[:],
                                start=True, stop=True,
                            )
                        for nt_i in range(NT):
                            nc.vector.reciprocal(
                                r1[:, nt_i:nt_i + 1], zs[:, 2 * nt_i:2 * nt_i + 1])
                            nc.vector.tensor_scalar_mul(
                                r2[:, nt_i:nt_i + 1], r1[:, nt_i:nt_i + 1], 2.0)

                    # ---- hop1: k-outer, full-width rhs --------------------
                    def hop1_epilogue(acc, nt_i, h):
                        st_bf = stream.tile([128, 512], BF16, tag="zstb")
                        nc.scalar.activation(
                            st_bf[:], acc[:], AF.Copy,
                            scale=r1[:, nt_i:nt_i + 1],
                        )
                        ag_dst = ag_in_a if h == 0 else ag_in_b
                        nc.gpsimd.dma_start(
                            ag_dst[nt_i * 128:(nt_i + 1) * 128, :], st_bf[:])
                        nc.gpsimd.dma_start(
                            scr1[h * 16:(h + 1) * 16, nt_i * 128:(nt_i + 1) * 128, :]
                            .transpose((1, 0, 2)),
                            st_bf[:].rearrange("p (c b) -> p c b", b=B),
                        )

                    with tc.tile_pool(name="ps_h1", bufs=1, space="PSUM") as ps_h1:
                        acc = [
                            ps_h1.tile([128, 512], F32, tag=f"acc{nt_i}_{h}",
                                       name=f"acc{nt_i}_{h}")
                            for nt_i in range(NT) for h in range(2)
                        ]
                        for k in range(MT):
                            rt = stream.tile([128, BC], BF16, tag="rhs", bufs=4)
                            nc.sync.dma_start(
                                rt[:], xt_bf[k * 128:(k + 1) * 128, :])
                            for nt_i in range(NT):
                                lhs = pt[:, k * NL + nt_i * 128: k * NL + (nt_i + 1) * 128]
                                for h in range(2):
                                    nc.tensor.matmul(
                                        acc[nt_i * 2 + h][:],
                                        lhs,
                                        rt[:, h * 512:(h + 1) * 512],
                                        start=(k == 0), stop=(k == MT - 1),
                                    )
                        for nt_i in range(NT):
                            for h in range(2):
                                hop1_epilogue(acc[nt_i * 2 + h], nt_i, h)

                    for ag_i, ag_o in ((ag_in_a, ag_out_a), (ag_in_b, ag_out_b)):
                        nc.gpsimd.collective_compute(
                            "AllGather",
                            mybir.AluOpType.bypass,
                            ins=[ag_i.opt()],
                            outs=[ag_o.opt()],
                            replica_groups=[list(range(NC_CORES))],
                        )

                    for ch in range(NT):
                        nc.gpsimd.dma_start(
                            xgs[ch][C:2 * C, :].rearrange("c (n b) -> c n b", b=B),
                            scr1[:, ch * 128:(ch + 1) * 128, :],
                        )

                    # weight-slab generation — fills the AllGather bubble on PE
                    nc.gpsimd.dma_start(
                        xloc_sb[:].rearrange("p (t f) -> p t f", f=BC),
                        xt_loc[:, :].rearrange("(t p) f -> p t f", p=128),
                    )
                    with tc.tile_pool(name="ps_wt", bufs=4, space="PSUM") as ps_wt:
                        for o in range(O):
                            w_ps = ps_wt.tile([96, NL], F32, tag="wps")
                            nc.tensor.matmul(
                                w_ps[:],
                                wpo_sb[:, o * (CHEB_K * C):(o + 1) * (CHEB_K * C)],
                                etl_bf[:],
                                start=True, stop=True,
                            )
                            src_v = w_ps[:].rearrange("p (nh nl) -> p nh nl", nl=8)
                            if o % 2 == 0:
                                nc.vector.tensor_copy(wt_i8[:, :, o, :], src_v)
                            else:
                                nc.scalar.activation(wt_i8[:, :, o, :], src_v, AF.Copy)

                    # ---- hop2: h-outer so the h=0 sweep overlaps AG_b -----
                    with tc.tile_pool(name="ps_h2", bufs=1, space="PSUM") as ps_h2:
                        acc2 = [
                            ps_h2.tile([128, 512], F32, tag=f"a2_{nt_i}_{h}",
                                       name=f"a2_{nt_i}_{h}")
                            for nt_i in range(NT) for h in range(2)
                        ]
                        for h, ag_o in ((0, ag_out_a), (1, ag_out_b)):
                            for k in range(MT):
                                rt2 = stream.tile([128, 512], BF16, tag="rhs2", bufs=4)
                                nc.sync.dma_start(
                                    rt2[:], ag_o[k * 128:(k + 1) * 128, :])
                                for nt_i in range(NT):
                                    nc.tensor.matmul(
                                        acc2[nt_i * 2 + h][:],
                                        pt[:, k * NL + nt_i * 128: k * NL + (nt_i + 1) * 128],
                                        rt2[:],
                                        start=(k == 0), stop=(k == MT - 1),
                                    )
                            for nt_i in range(NT):
                                a = acc2[nt_i * 2 + h]
                                st = stream.tile([128, 512], F32, tag="zst", bufs=2)
                                nc.scalar.activation(
                                    st[:], a[:], AF.Copy,
                                    scale=r2[:, nt_i:nt_i + 1],
                                )
                                st_bf = stream.tile([128, 512], BF16, tag="zstb")
                                nc.vector.tensor_tensor(
                                    st_bf[:], st[:],
                                    xloc_sb[:, nt_i * BC + h * 512: nt_i * BC + (h + 1) * 512],
                                    mybir.AluOpType.subtract,
                                )
                                nc.gpsimd.dma_start(
                                    scr2[h * 16:(h + 1) * 16, nt_i * 128:(nt_i + 1) * 128, :]
                                    .transpose((1, 0, 2)),
                                    st_bf[:].rearrange("p (c b) -> p c b", b=B),
                                )

                    for ch in range(NT):
                        eng = nc.sync if ch % 2 == 0 else nc.scalar
                        eng.dma_start(
                            xgs[ch][2 * C:3 * C, :].rearrange("c (n b) -> c n b", b=B),
                            scr2[:, ch * 128:(ch + 1) * 128, :],
                        )
                        nc.vector.memset(xgs[ch][96:97, :], 1.0)

                # ---- grouped per-node GEMM (col-tiled, 4 nodes concurrent) ----
                with tc.tile_pool(name="ps_g", bufs=8, space="PSUM") as ps_g:
                    for ch in range(NT):  # 128 nodes per chunk
                        n0 = ch * 128
                        xg_b = xgs[ch]
                        for q16 in range(4):  # 32 nodes per psum tile
                            g_ps = ps_g.tile([128, 512], F32, tag="gps")
                            for j in range(8):
                                for g in range(4):
                                    nl_i = q16 * 32 + j * 4 + g
                                    n_gl = n0 + nl_i
                                    nc.tensor.matmul(
                                        g_ps[32 * g:32 * (g + 1), j * O:(j + 1) * O],
                                        xg_b[:, nl_i * B:(nl_i + 1) * B],
                                        wt_g[:, n_gl // 8, n_gl % 8, :],
                                        start=True, stop=True,
                                        tile_position=(0, 32 * g),
                                    )
                            st = tstream.tile([128, 512], F32, tag="gst", bufs=2)
                            nc.vector.tensor_copy(st[:], g_ps[:])
                            dst4 = out_loc[:, n0 + q16 * 32: n0 + (q16 + 1) * 32, :] \
                                .rearrange("b (j g) o -> g b j o", g=4)
                            for g in range(4):
                                eng = nc.sync if g % 2 == 0 else nc.gpsimd
                                eng.dma_start(
                                    dst4[g],
                                    st[32 * g:32 * (g + 1), :].rearrange(
                                        "b (j o) -> b j o", o=O),
                                )
                xgp_cm.__exit__(None, None, None)
    return out_loc


_COMPILED = None


def _get_compiled():
    global _COMPILED
    if _COMPILED is None:
        nc = bacc.Bacc(
            "TRN2",
            target_bir_lowering=False,
            debug=False,
            num_devices=NC_CORES,
        )
        _build(nc)
        nc.compile()
        _COMPILED = nc
    return _COMPILED


def kernel(x, node_embeddings, laplacian_mx, weights_pool, bias_pool):
    x = np.asarray(x, dtype=np.float32)
    e = np.asarray(node_embeddings, dtype=np.float32)
    wp = np.asarray(weights_pool, dtype=np.float32)
    bp = np.asarray(bias_pool, dtype=np.float32)

    et = np.ascontiguousarray(e.T)                                  # [D, N]
    xt_h = np.ascontiguousarray(x.transpose(1, 2, 0).reshape(N, BC))  # [m, c*32+b]
    wpo = np.ascontiguousarray(wp.transpose(3, 0, 1, 2).reshape(O, D, CHEB_K * C)).astype(ml_dtypes.bfloat16)
    bias_h = (e @ bp).astype(np.float32)                            # [N, O]

    xt_b = xt_h.astype(ml_dtypes.bfloat16)
    in_maps = []
    for i in range(NC_CORES):
        sl = slice(i * NL, (i + 1) * NL)
        in_maps.append({
            "et": et,
            "et_loc": np.ascontiguousarray(et[:, sl]),
            "xt_bf": xt_b,
            "xtl_bf": np.ascontiguousarray(xt_b[sl]),
            "xt_loc": np.ascontiguousarray(xt_h[sl]),
            "wp_o": wpo,
            "bias_flat": np.ascontiguousarray(
                bias_h[sl].reshape(64, 8, O).transpose(0, 2, 1).reshape(1, NL * O).astype(ml_dtypes.bfloat16)
            ),
        })

    nc = _get_compiled()
    trace = bool(int(os.environ.get("KBENCH_TRACE", "0")))
    if trace:
        trace = _register_ntff_hook()
    res = run_bass_kernel_spmd(
        nc,
        in_maps,
        core_ids=list(range(NC_CORES)),
        trace=trace,
    )
    LAST_RESULTS["exec_time_ns"] = res.exec_time_ns
    LAST_RESULTS["trace"] = res.instructions_and_trace
    LAST_RESULTS["mean_exec_time_ns"] = res.mean_exec_time_ns

    out = np.empty((B, N, O), dtype=np.float32)
    for i in range(NC_CORES):
        out[:, i * NL:(i + 1) * NL, :] = res.results[i]["out_loc"]
    return out


# revision 30
# speedup vs baseline: 1.1260x; 1.1260x over previous
"""Trainium2 Bass kernel for nn_Embedded_GCN (gnn_message_passing).

Reference math (B=32, N=4096, C=32, O=64, D=16, K=3):
  A  = softmax(relu(E @ E.T), axis=1)              # [N, N] adaptive adjacency
  T0 = I, T1 = A, T2 = 2A@A - I                    # Chebyshev
  x_g[k]   = T_k @ x_b  for each batch             # [B, K, N, C]
  W[n]     = sum_d E[n,d] * Wp[d]                  # per-node weights [K,C,O]
  out[b,n] = sum_{k,i} x_g[b,n,k,i] W[n,k,i,:] + E[n]@bias_pool

Key restructurings:
  * T2 is never materialized: z1 = A@x, z2 = 2*A@z1 - x (avoids the N^3 matmul).
  * softmax(relu(s)) = max(1, exp(s)) / rowsum  (exp never overflows: logits <~ 50).
  * Scores are computed directly transposed: PT[m, n] = exp-scores with the
    contraction (m) on partitions, so the two propagation hops need no on-chip
    transpose of the 4096x4096 attention matrix.
  * Row sums via ones-matmul on the PE; 1/Z folded into the hop PSUM->SBUF
    epilogue as a per-partition activation scale.
  * Big matmuls run in float32r (FP22 truncation, full PE speed at N>=512).
  * Node-sharding across 8 cores (512 nodes each); z1 is AllGathered (2MB/rank).
  * Per-node grouped GEMM: x_g is permuted to [(k,c), (n,b)] via a DRAM
    round-trip (contiguous-ish chunks both directions), per-node weights are
    generated on the PE into a [97, (n,o)] slab (97th row = bias, matched by a
    ones-row in x_gT), and 512 small [97,32]x[97,64] bf16 matmuls finish it.
"""

import os

import numpy as np
import ml_dtypes

import concourse.bass as bass
import concourse.mybir as mybir
import concourse.tile as tile
from concourse import bacc
from concourse.bass_utils import run_bass_kernel_spmd

F32 = mybir.dt.float32
F32R = mybir.dt.float32r
BF16 = mybir.dt.bfloat16
AF = mybir.ActivationFunctionType

B, N, C, O, D, CHEB_K = 32, 4096, 32, 64, 16, 3
NC_CORES = 8
NL = N // NC_CORES          # 512 nodes per core
BC = B * C                  # 1024
MT = N // 128               # 32 contraction tiles
NT = NL // 128              # 4 local node tiles

LAST_RESULTS = {}


def _register_ntff_hook():
    """Inject antenv.axon_hooks (absent from the container's antenv stub) and
    register the ctypes NTFF-profile hook so trace=True works under axon."""
    import sys
    import types

    try:
        import antenv

        if "antenv.axon_hooks" not in sys.modules:
            mod = types.ModuleType("antenv.axon_hooks")
            mod._hook = None

            def set_axon_ntff_profile_hook(h):
                mod._hook = h

            def get_axon_ntff_profile_hook():
                return mod._hook

            mod.set_axon_ntff_profile_hook = set_axon_ntff_profile_hook
            mod.get_axon_ntff_profile_hook = get_axon_ntff_profile_hook
            sys.modules["antenv.axon_hooks"] = mod
            antenv.axon_hooks = mod

        hooks = sys.modules["antenv.axon_hooks"]
        if hooks.get_axon_ntff_profile_hook() is None:
            from trn_agent_boot.trn_boot import _ntff_profile_via_ctypes

            hook = _ntff_profile_via_ctypes("/opt/axon/libaxon_pjrt.so")
            if hook is not None:
                hooks.set_axon_ntff_profile_hook(hook)
        return True
    except Exception:
        return False


def _build(nc: bacc.Bacc):
    # ---- I/O -------------------------------------------------------------
    et = nc.dram_tensor("et", [D, N], F32, kind="ExternalInput")          # E^T
    et_loc = nc.dram_tensor("et_loc", [D, NL], F32, kind="ExternalInput")
    xt_bf = nc.dram_tensor("xt_bf", [N, BC], BF16, kind="ExternalInput")   # x[b,m,c] -> [m, c*32+b]
    xtl_bf = nc.dram_tensor("xtl_bf", [NL, BC], BF16, kind="ExternalInput")
    xt_loc = nc.dram_tensor("xt_loc", [NL, BC], F32, kind="ExternalInput")
    wp_o = nc.dram_tensor("wp_o", [O, D, CHEB_K * C], BF16, kind="ExternalInput")
    bias_flat = nc.dram_tensor("bias_flat", [1, NL * O], BF16, kind="ExternalInput")
    out_loc = nc.dram_tensor("out_loc", [B, NL, O], F32, kind="ExternalOutput")

    with tile.TileContext(nc) as tc:
        with tc.tile_pool(name="dram", bufs=1, space="DRAM") as dram, \
             tc.tile_pool(name="persist", bufs=1) as persist:

            ag_in_a = dram.tile([NL, 512], BF16, tag="ag_in_a")
            ag_in_b = dram.tile([NL, 512], BF16, tag="ag_in_b")
            ag_out_a = dram.tile([N, 512], BF16, tag="ag_out_a", addr_space="Shared")
            ag_out_b = dram.tile([N, 512], BF16, tag="ag_out_b", addr_space="Shared")
            scr1 = dram.tile([C, NL, B], BF16, tag="scr1")   # z1 as [c, n, b]
            scr2 = dram.tile([C, NL, B], BF16, tag="scr2")   # z2 as [c, n, b]
            r_dram = dram.tile([2, NL], F32, tag="r_dram")

            # ---- small persistent SBUF ------------------------------------
            etl_sb = persist.tile([D, NL], F32R, tag="etl")
            r1 = persist.tile([128, NT], F32, tag="r1")          # 1/Z  per node col nt
            r2 = persist.tile([128, NT], F32, tag="r2")          # 2/Z

            etl_bf = persist.tile([D, NL], BF16, tag="etlbf")
            nc.sync.dma_start(etl_sb[:], et_loc[:, :].bitcast(F32R))
            nc.vector.tensor_copy(etl_bf[:], etl_sb[:].bitcast(F32))

            with tc.tile_pool(name="wtp", bufs=1) as wtp, \
                 tc.tile_pool(name="tstream", bufs=3) as tstream:
                # weight slab, n-major: [ki|bias, (n, o)]
                wt_bf = wtp.tile([97, NL * O], BF16, tag="wt")
                nc.gpsimd.dma_start(wt_bf[96:97, :], bias_flat[:, :])
                # interleaved: free idx = n_hi*512 + o*8 + n_lo, node n = n_hi*8 + n_lo
                wt_i8 = wt_bf[0:96, :].rearrange("p (nh o nl) -> p nh o nl", o=O, nl=8)
                wt_g = wt_bf[:].rearrange("p (nh o nl) -> p nh nl o", o=O, nl=8)

                xgp_cm = tc.tile_pool(name="xg", bufs=1)
                xgp = xgp_cm.__enter__()
                xgs = []
                for ch in range(NT):
                    n0 = ch * 128
                    xg_t = xgp.tile([97, 128 * B], BF16, tag=f"xg{ch}", name=f"xg{ch}")
                    nc.gpsimd.dma_start(
                        xg_t[0:C, :].rearrange("c (n b) -> c n b", b=B),
                        xtl_bf[n0:n0 + 128, :].rearrange("n (c b) -> c n b", b=B),
                    )
                    xgs.append(xg_t)

                wpo_sb = wtp.tile([D, O * CHEB_K * C], BF16, tag="wpo")
                nc.scalar.dma_start(
                    wpo_sb[:].rearrange("d (o k) -> d o k", k=CHEB_K * C),
                    wp_o[:, :, :].transpose((1, 0, 2)),
                )

                with tc.tile_pool(name="ptp", bufs=1) as ptp, \
                     tc.tile_pool(name="stream", bufs=3) as stream:
                    pt = ptp.tile([128, MT * NL], BF16, tag="pt")  # PT[m%128, mt*NL+n]
                    xloc_sb = ptp.tile([128, NT * BC], F32, tag="xloc")

                    # ---- phase B/C: transposed exp-scores + row sums ------
                    ones_f = persist.tile([128, 2], F32, tag="onesf")
                    nc.vector.memset(ones_f[:], 1.0)
                    with tc.tile_pool(name="etp", bufs=2) as etp, \
                         tc.tile_pool(name="ps_sc", bufs=3, space="PSUM") as ps_sc:
                        # interleaved accumulators: short critical path after last exp
                        accs = [ptp.tile([128, NL], F32, tag=f"accs{i}", name=f"accs{i}")
                                for i in range(2)]
                        et_c = None
                        for mt in range(MT):
                            if mt % 8 == 0:
                                et_c = etp.tile([D, 1024], F32R, tag="etc")
                                nc.sync.dma_start(
                                    et_c[:],
                                    et[:, mt * 128:(mt + 8) * 128].bitcast(F32R))
                            s_ps = ps_sc.tile([128, NL], F32, tag="s")
                            nc.tensor.matmul(
                                s_ps[:],
                                et_c[:, (mt % 8) * 128:(mt % 8 + 1) * 128],
                                etl_sb[:],
                                start=True, stop=True,
                            )
                            pslice = pt[:, mt * NL:(mt + 1) * NL]
                            nc.scalar.activation(pslice, s_ps[:], AF.Exp)
                            nc.vector.tensor_scalar_max(pslice, pslice, 1.0)
                            a = accs[mt % 2]
                            if mt < 2:
                                nc.vector.tensor_copy(a[:], pslice)
                            else:
                                nc.vector.tensor_tensor(
                                    a[:], a[:], pslice, mybir.AluOpType.add)
                        nc.vector.tensor_tensor(
                            accs[0][:], accs[0][:], accs[1][:], mybir.AluOpType.add)
                        acc_fin = accs[0]

                    # ---- hop1: k-outer, full-width rhs --------------------
                    with tc.tile_pool(name="ps_h1", bufs=1, space="PSUM") as ps_h1:
                        acc = [
                            ps_h1.tile([128, 512], F32, tag=f"acc{nt_i}_{h}",
                                       name=f"acc{nt_i}_{h}")
                            for nt_i in range(NT) for h in range(2)
                        ]
                        for k in range(MT):
                            rt = stream.tile([128, BC], BF16, tag="rhs", bufs=3)
                            nc.sync.dma_start(
                                rt[:], xt_bf[k * 128:(k + 1) * 128, :])
                            for nt_i in range(NT):
                                lhs = pt[:, k * NL + nt_i * 128: k * NL + (nt_i + 1) * 128]
                                for h in range(2):
                                    nc.tensor.matmul(
                                        acc[nt_i * 2 + h][:],
                                        lhs,
                                        rt[:, h * 512:(h + 1) * 512],
                                        start=(k == 0), stop=(k == MT - 1),
                                    )
                        # row-sums: save acc0 raw, reuse its bank for the zs matmuls
                        raw0 = stream.tile([128, 512], F32, tag="raw0", bufs=1)
                        nc.scalar.activation(raw0[:], acc[0][:], AF.Copy)
                        zs = ps_h1.tile([128, 8], F32, tag="acc0_0", name="zs")
                        for nt_i in range(NT):
                            nc.tensor.matmul(
                                zs[:, 2 * nt_i:2 * nt_i + 2],
                                acc_fin[:, nt_i * 128:(nt_i + 1) * 128],
                                ones_f[:],
                                start=True, stop=True,
                            )
                        for nt_i in range(NT):
                            nc.vector.reciprocal(
                                r1[:, nt_i:nt_i + 1], zs[:, 2 * nt_i:2 * nt_i + 1])
                            nc.vector.tensor_scalar_mul(
                                r2[:, nt_i:nt_i + 1], r1[:, nt_i:nt_i + 1], 2.0)

                        # epilogues: scale on DVE; ag stores first
                        sts = []
                        for nt_i in range(NT):
                            for h in range(2):
                                st_bf = stream.tile([128, 512], BF16, tag=f"zb{nt_i}_{h}",
                                                    name=f"zb{nt_i}_{h}", bufs=1)
                                src_ap = raw0[:] if (nt_i == 0 and h == 0) else acc[nt_i * 2 + h][:]
                                nc.vector.tensor_scalar(
                                    st_bf[:], src_ap, r1[:, nt_i:nt_i + 1], None,
                                    op0=mybir.AluOpType.mult,
                                )
                                ag_dst = ag_in_a if h == 0 else ag_in_b
                                nc.gpsimd.dma_start(
                                    ag_dst[nt_i * 128:(nt_i + 1) * 128, :], st_bf[:])
                                sts.append((st_bf, nt_i, h))

                    for ag_i, ag_o in ((ag_in_a, ag_out_a), (ag_in_b, ag_out_b)):
                        nc.gpsimd.collective_compute(
                            "AllGather",
                            mybir.AluOpType.bypass,
                            ins=[ag_i.opt()],
                            outs=[ag_o.opt()],
                            replica_groups=[list(range(NC_CORES))],
                        )

                    for st_bf, nt_i, h in sts:
                        nc.gpsimd.dma_start(
                            scr1[h * 16:(h + 1) * 16, nt_i * 128:(nt_i + 1) * 128, :]
                            .transpose((1, 0, 2)),
                            st_bf[:].rearrange("p (c b) -> p c b", b=B),
                        )

                    for ch in range(NT):
                        nc.gpsimd.dma_start(
                            xgs[ch][C:2 * C, :].rearrange("c (n b) -> c n b", b=B),
                            scr1[:, ch * 128:(ch + 1) * 128, :],
                        )

                    # weight-slab generation — fills the AllGather bubble on PE
                    nc.gpsimd.dma_start(
                        xloc_sb[:].rearrange("p (t f) -> p t f", f=BC),
                        xt_loc[:, :].rearrange("(t p) f -> p t f", p=128),
                    )
                    with tc.tile_pool(name="ps_wt", bufs=4, space="PSUM") as ps_wt:
                        for o in range(O):
                            w_ps = ps_wt.tile([96, NL], F32, tag="wps")
                            nc.tensor.matmul(
                                w_ps[:],
                                wpo_sb[:, o * (CHEB_K * C):(o + 1) * (CHEB_K * C)],
                                etl_bf[:],
                                start=True, stop=True,
                            )
                            src_v = w_ps[:].rearrange("p (nh nl) -> p nh nl", nl=8)
                            if o % 2 == 0:
                                nc.vector.tensor_copy(wt_i8[:, :, o, :], src_v)
                            else:
                                nc.scalar.activation(wt_i8[:, :, o, :], src_v, AF.Copy)

                    # ---- hop2: h-outer so the h=0 sweep overlaps AG_b -----
                    with tc.tile_pool(name="ps_h2", bufs=1, space="PSUM") as ps_h2:
                        acc2 = [
                            ps_h2.tile([128, 512], F32, tag=f"a2_{nt_i}_{h}",
                                       name=f"a2_{nt_i}_{h}")
                            for nt_i in range(NT) for h in range(2)
                        ]
                        for h, ag_o in ((0, ag_out_a), (1, ag_out_b)):
                            for k in range(MT):
                                rt2 = stream.tile([128, 512], BF16, tag="rhs2", bufs=4)
                                nc.sync.dma_start(
                                    rt2[:], ag_o[k * 128:(k + 1) * 128, :])
                                for nt_i in range(NT):
                                    nc.tensor.matmul(
                                        acc2[nt_i * 2 + h][:],
                                        pt[:, k * NL + nt_i * 128: k * NL + (nt_i + 1) * 128],
                                        rt2[:],
                                        start=(k == 0), stop=(k == MT - 1),
                                    )
                            for nt_i in range(NT):
                                a = acc2[nt_i * 2 + h]
                                st = stream.tile([128, 512], F32, tag="zst", bufs=2)
                                nc.scalar.activation(
                                    st[:], a[:], AF.Copy,
                                    scale=r2[:, nt_i:nt_i + 1],
                                )
                                st_bf = stream.tile([128, 512], BF16, tag="zstb")
                                nc.vector.tensor_tensor(
                                    st_bf[:], st[:],
                                    xloc_sb[:, nt_i * BC + h * 512: nt_i * BC + (h + 1) * 512],
                                    mybir.AluOpType.subtract,
                                )
                                nc.gpsimd.dma_start(
                                    scr2[h * 16:(h + 1) * 16, nt_i * 128:(nt_i + 1) * 128, :]
                                    .transpose((1, 0, 2)),
                                    st_bf[:].rearrange("p (c b) -> p c b", b=B),
                                )

                    for ch in range(NT):
                        eng = nc.sync if ch % 2 == 0 else nc.scalar
                        eng.dma_start(
                            xgs[ch][2 * C:3 * C, :].rearrange("c (n b) -> c n b", b=B),
                            scr2[:, ch * 128:(ch + 1) * 128, :],
                        )
                        nc.vector.memset(xgs[ch][96:97, :], 1.0)

                # ---- grouped per-node GEMM (col-tiled, 4 nodes concurrent) ----
                with tc.tile_pool(name="ps_g", bufs=8, space="PSUM") as ps_g:
                    for ch in range(NT):  # 128 nodes per chunk
                        n0 = ch * 128
                        xg_b = xgs[ch]
                        for q16 in range(4):  # 32 nodes per psum tile
                            g_ps = ps_g.tile([128, 512], F32, tag="gps")
                            for j in range(8):
                                for g in range(4):
                                    nl_i = q16 * 32 + j * 4 + g
                                    n_gl = n0 + nl_i
                                    nc.tensor.matmul(
                                        g_ps[32 * g:32 * (g + 1), j * O:(j + 1) * O],
                                        xg_b[:, nl_i * B:(nl_i + 1) * B],
                                        wt_g[:, n_gl // 8, n_gl % 8, :],
                                        start=True, stop=True,
                                        tile_position=(0, 32 * g),
                                    )
                            st = tstream.tile([128, 512], F32, tag="gst", bufs=2)
                            nc.vector.tensor_copy(st[:], g_ps[:])
                            dst4 = out_loc[:, n0 + q16 * 32: n0 + (q16 + 1) * 32, :] \
                                .rearrange("b (j g) o -> g b j o", g=4)
                            for g in range(4):
                                eng = nc.sync if g % 2 == 0 else nc.gpsimd
                                eng.dma_start(
                                    dst4[g],
                                    st[32 * g:32 * (g + 1), :].rearrange(
                                        "b (j o) -> b j o", o=O),
                                )
                xgp_cm.__exit__(None, None, None)
    return out_loc


_COMPILED = None


def _get_compiled():
    global _COMPILED
    if _COMPILED is None:
        nc = bacc.Bacc(
            "TRN2",
            target_bir_lowering=False,
            debug=False,
            num_devices=NC_CORES,
        )
        _build(nc)
        nc.compile()
        _COMPILED = nc
    return _COMPILED


def kernel(x, node_embeddings, laplacian_mx, weights_pool, bias_pool):
    x = np.asarray(x, dtype=np.float32)
    e = np.asarray(node_embeddings, dtype=np.float32)
    wp = np.asarray(weights_pool, dtype=np.float32)
    bp = np.asarray(bias_pool, dtype=np.float32)

    et = np.ascontiguousarray(e.T)                                  # [D, N]
    xt_h = np.ascontiguousarray(x.transpose(1, 2, 0).reshape(N, BC))  # [m, c*32+b]
    wpo = np.ascontiguousarray(wp.transpose(3, 0, 1, 2).reshape(O, D, CHEB_K * C)).astype(ml_dtypes.bfloat16)
    bias_h = (e @ bp).astype(np.float32)                            # [N, O]

    xt_b = xt_h.astype(ml_dtypes.bfloat16)
    in_maps = []
    for i in range(NC_CORES):
        sl = slice(i * NL, (i + 1) * NL)
        in_maps.append({
            "et": et,
            "et_loc": np.ascontiguousarray(et[:, sl]),
            "xt_bf": xt_b,
            "xtl_bf": np.ascontiguousarray(xt_b[sl]),
            "xt_loc": np.ascontiguousarray(xt_h[sl]),
            "wp_o": wpo,
            "bias_flat": np.ascontiguousarray(
                bias_h[sl].reshape(64, 8, O).transpose(0, 2, 1).reshape(1, NL * O).astype(ml_dtypes.bfloat16)
            ),
        })

    nc = _get_compiled()
    trace = bool(int(os.environ.get("KBENCH_TRACE", "0")))
    if trace:
        trace = _register_ntff_hook()
    res = run_bass_kernel_spmd(
        nc,
        in_maps,
        core_ids=list(range(NC_CORES)),
        trace=trace,
    )
    LAST_RESULTS["exec_time_ns"] = res.exec_time_ns
    LAST_RESULTS["trace"] = res.instructions_and_trace
    LAST_RESULTS["mean_exec_time_ns"] = res.mean_exec_time_ns

    out = np.empty((B, N, O), dtype=np.float32)
    for i in range(NC_CORES):
        out[:, i * NL:(i + 1) * NL, :] = res.results[i]["out_loc"]
    return out


# revision 31
# speedup vs baseline: 1.1368x; 1.0096x over previous
"""Trainium2 Bass kernel for nn_Embedded_GCN (gnn_message_passing).

Reference math (B=32, N=4096, C=32, O=64, D=16, K=3):
  A  = softmax(relu(E @ E.T), axis=1)              # [N, N] adaptive adjacency
  T0 = I, T1 = A, T2 = 2A@A - I                    # Chebyshev
  x_g[k]   = T_k @ x_b  for each batch             # [B, K, N, C]
  W[n]     = sum_d E[n,d] * Wp[d]                  # per-node weights [K,C,O]
  out[b,n] = sum_{k,i} x_g[b,n,k,i] W[n,k,i,:] + E[n]@bias_pool

Key restructurings:
  * T2 is never materialized: z1 = A@x, z2 = 2*A@z1 - x (avoids the N^3 matmul).
  * softmax(relu(s)) = max(1, exp(s)) / rowsum  (exp never overflows: logits <~ 50).
  * Scores are computed directly transposed: PT[m, n] = exp-scores with the
    contraction (m) on partitions, so the two propagation hops need no on-chip
    transpose of the 4096x4096 attention matrix.
  * Row sums via ones-matmul on the PE; 1/Z folded into the hop PSUM->SBUF
    epilogue as a per-partition activation scale.
  * Big matmuls run in float32r (FP22 truncation, full PE speed at N>=512).
  * Node-sharding across 8 cores (512 nodes each); z1 is AllGathered (2MB/rank).
  * Per-node grouped GEMM: x_g is permuted to [(k,c), (n,b)] via a DRAM
    round-trip (contiguous-ish chunks both directions), per-node weights are
    generated on the PE into a [97, (n,o)] slab (97th row = bias, matched by a
    ones-row in x_gT), and 512 small [97,32]x[97,64] bf16 matmuls finish it.
"""

import os

import numpy as np
import ml_dtypes

import concourse.bass as bass
import concourse.mybir as mybir
import concourse.tile as tile
from concourse import bacc
from concourse.bass_utils import run_bass_kernel_spmd

F32 = mybir.dt.float32
F32R = mybir.dt.float32r
BF16 = mybir.dt.bfloat16
AF = mybir.ActivationFunctionType

B, N, C, O, D, CHEB_K = 32, 4096, 32, 64, 16, 3
NC_CORES = 8
NL = N // NC_CORES          # 512 nodes per core
BC = B * C                  # 1024
MT = N // 128               # 32 contraction tiles
NT = NL // 128              # 4 local node tiles

LAST_RESULTS = {}


def _register_ntff_hook():
    """Inject antenv.axon_hooks (absent from the container's antenv stub) and
    register the ctypes NTFF-profile hook so trace=True works under axon."""
    import sys
    import types

    try:
        import antenv

        if "antenv.axon_hooks" not in sys.modules:
            mod = types.ModuleType("antenv.axon_hooks")
            mod._hook = None

            def set_axon_ntff_profile_hook(h):
                mod._hook = h

            def get_axon_ntff_profile_hook():
                return mod._hook

            mod.set_axon_ntff_profile_hook = set_axon_ntff_profile_hook
            mod.get_axon_ntff_profile_hook = get_axon_ntff_profile_hook
            sys.modules["antenv.axon_hooks"] = mod
            antenv.axon_hooks = mod

        hooks = sys.modules["antenv.axon_hooks"]
        if hooks.get_axon_ntff_profile_hook() is None:
            from trn_agent_boot.trn_boot import _ntff_profile_via_ctypes

            hook = _ntff_profile_via_ctypes("/opt/axon/libaxon_pjrt.so")
            if hook is not None:
                hooks.set_axon_ntff_profile_hook(hook)
        return True
    except Exception:
        return False


def _build(nc: bacc.Bacc):
    # ---- I/O -------------------------------------------------------------
    et = nc.dram_tensor("et", [D, N], F32, kind="ExternalInput")          # E^T
    et_loc = nc.dram_tensor("et_loc", [D, NL], F32, kind="ExternalInput")
    xt_bf = nc.dram_tensor("xt_bf", [N, BC], BF16, kind="ExternalInput")   # x[b,m,c] -> [m, c*32+b]
    xtl_bf = nc.dram_tensor("xtl_bf", [NL, BC], BF16, kind="ExternalInput")
    xt_loc = nc.dram_tensor("xt_loc", [NL, BC], F32, kind="ExternalInput")
    wp_o = nc.dram_tensor("wp_o", [O, D, CHEB_K * C], BF16, kind="ExternalInput")
    bias_flat = nc.dram_tensor("bias_flat", [1, NL * O], BF16, kind="ExternalInput")
    out_loc = nc.dram_tensor("out_loc", [B, NL, O], F32, kind="ExternalOutput")

    with tile.TileContext(nc) as tc:
        with tc.tile_pool(name="dram", bufs=1, space="DRAM") as dram, \
             tc.tile_pool(name="persist", bufs=1) as persist:

            ag_ins = [dram.tile([128, BC], BF16, tag=f"ag_in{q}", name=f"ag_in{q}")
                      for q in range(NT)]
            ag_outs = [dram.tile([NC_CORES * 128, BC], BF16, tag=f"ag_out{q}",
                                 name=f"ag_out{q}", addr_space="Shared")
                       for q in range(NT)]
            scr1 = dram.tile([C, NL, B], BF16, tag="scr1")   # z1 as [c, n, b]
            scr2 = dram.tile([C, NL, B], BF16, tag="scr2")   # z2 as [c, n, b]
            r_dram = dram.tile([2, NL], F32, tag="r_dram")

            # ---- small persistent SBUF ------------------------------------
            etl_sb = persist.tile([D, NL], F32R, tag="etl")
            r1 = persist.tile([128, NT], F32, tag="r1")          # 1/Z  per node col nt
            r2 = persist.tile([128, NT], F32, tag="r2")          # 2/Z

            etl_bf = persist.tile([D, NL], BF16, tag="etlbf")
            nc.sync.dma_start(etl_sb[:], et_loc[:, :].bitcast(F32R))
            nc.vector.tensor_copy(etl_bf[:], etl_sb[:].bitcast(F32))

            with tc.tile_pool(name="wtp", bufs=1) as wtp, \
                 tc.tile_pool(name="tstream", bufs=3) as tstream:
                # weight slab, n-major: [ki|bias, (n, o)]
                wt_bf = wtp.tile([97, NL * O], BF16, tag="wt")
                nc.gpsimd.dma_start(wt_bf[96:97, :], bias_flat[:, :])
                # interleaved: free idx = n_hi*512 + o*8 + n_lo, node n = n_hi*8 + n_lo
                wt_i8 = wt_bf[0:96, :].rearrange("p (nh o nl) -> p nh o nl", o=O, nl=8)
                wt_g = wt_bf[:].rearrange("p (nh o nl) -> p nh nl o", o=O, nl=8)

                xgp_cm = tc.tile_pool(name="xg", bufs=1)
                xgp = xgp_cm.__enter__()
                xgs = []
                for ch in range(NT):
                    n0 = ch * 128
                    xg_t = xgp.tile([97, 128 * B], BF16, tag=f"xg{ch}", name=f"xg{ch}")
                    nc.gpsimd.dma_start(
                        xg_t[0:C, :].rearrange("c (n b) -> c n b", b=B),
                        xtl_bf[n0:n0 + 128, :].rearrange("n (c b) -> c n b", b=B),
                    )
                    xgs.append(xg_t)

                wpo_sb = wtp.tile([D, O * CHEB_K * C], BF16, tag="wpo")
                nc.scalar.dma_start(
                    wpo_sb[:].rearrange("d (o k) -> d o k", k=CHEB_K * C),
                    wp_o[:, :, :].transpose((1, 0, 2)),
                )

                with tc.tile_pool(name="ptp", bufs=1) as ptp, \
                     tc.tile_pool(name="stream", bufs=3) as stream:
                    pt = ptp.tile([128, MT * NL], BF16, tag="pt")  # PT[m%128, mt*NL+n]
                    xloc_sb = ptp.tile([128, NT * BC], F32, tag="xloc")

                    # ---- phase B/C: transposed exp-scores + row sums ------
                    ones_f = persist.tile([128, 2], F32, tag="onesf")
                    nc.vector.memset(ones_f[:], 1.0)
                    with tc.tile_pool(name="etp", bufs=2) as etp, \
                         tc.tile_pool(name="ps_sc", bufs=3, space="PSUM") as ps_sc:
                        # interleaved accumulators: short critical path after last exp
                        accs = [ptp.tile([128, NL], F32, tag=f"accs{i}", name=f"accs{i}")
                                for i in range(2)]
                        et_c = None
                        for mt in range(MT):
                            if mt % 8 == 0:
                                et_c = etp.tile([D, 1024], F32R, tag="etc")
                                nc.sync.dma_start(
                                    et_c[:],
                                    et[:, mt * 128:(mt + 8) * 128].bitcast(F32R))
                            s_ps = ps_sc.tile([128, NL], F32, tag="s")
                            nc.tensor.matmul(
                                s_ps[:],
                                et_c[:, (mt % 8) * 128:(mt % 8 + 1) * 128],
                                etl_sb[:],
                                start=True, stop=True,
                            )
                            pslice = pt[:, mt * NL:(mt + 1) * NL]
                            nc.scalar.activation(pslice, s_ps[:], AF.Exp)
                            nc.vector.tensor_scalar_max(pslice, pslice, 1.0)
                            a = accs[mt % 2]
                            if mt < 2:
                                nc.vector.tensor_copy(a[:], pslice)
                            else:
                                nc.vector.tensor_tensor(
                                    a[:], a[:], pslice, mybir.AluOpType.add)
                        nc.vector.tensor_tensor(
                            accs[0][:], accs[0][:], accs[1][:], mybir.AluOpType.add)
                        acc_fin = accs[0]

                    for ch in range(NT):
                        nc.vector.memset(xgs[ch][96:97, :], 1.0)

                    # ---- hop1: k-outer, full-width rhs --------------------
                    with tc.tile_pool(name="ps_h1", bufs=1, space="PSUM") as ps_h1:
                        acc = [
                            ps_h1.tile([128, 512], F32, tag=f"acc{nt_i}_{h}",
                                       name=f"acc{nt_i}_{h}")
                            for nt_i in range(NT) for h in range(2)
                        ]
                        for k in range(MT):
                            rt = stream.tile([128, BC], BF16, tag="rhs", bufs=3)
                            nc.sync.dma_start(
                                rt[:], xt_bf[k * 128:(k + 1) * 128, :])
                            for nt_i in range(NT):
                                lhs = pt[:, k * NL + nt_i * 128: k * NL + (nt_i + 1) * 128]
                                for h in range(2):
                                    nc.tensor.matmul(
                                        acc[nt_i * 2 + h][:],
                                        lhs,
                                        rt[:, h * 512:(h + 1) * 512],
                                        start=(k == 0), stop=(k == MT - 1),
                                    )
                        # row-sums: save acc0 raw, reuse its bank for the zs matmuls
                        raw0 = stream.tile([128, 512], F32, tag="raw0", bufs=1)
                        nc.scalar.activation(raw0[:], acc[0][:], AF.Copy)
                        zs = ps_h1.tile([128, 8], F32, tag="acc0_0", name="zs")
                        for nt_i in range(NT):
                            nc.tensor.matmul(
                                zs[:, 2 * nt_i:2 * nt_i + 2],
                                acc_fin[:, nt_i * 128:(nt_i + 1) * 128],
                                ones_f[:],
                                start=True, stop=True,
                            )
                        for nt_i in range(NT):
                            nc.vector.reciprocal(
                                r1[:, nt_i:nt_i + 1], zs[:, 2 * nt_i:2 * nt_i + 1])
                            nc.vector.tensor_scalar_mul(
                                r2[:, nt_i:nt_i + 1], r1[:, nt_i:nt_i + 1], 2.0)

                        # epilogues: scale on DVE; per-nt AG triggered as soon
                        # as its slice is stored
                        sts = []
                        for nt_i in range(NT):
                            for h in range(2):
                                st_bf = stream.tile([128, 512], BF16, tag=f"zb{nt_i}_{h}",
                                                    name=f"zb{nt_i}_{h}", bufs=1)
                                src_ap = raw0[:] if (nt_i == 0 and h == 0) else acc[nt_i * 2 + h][:]
                                nc.vector.tensor_scalar(
                                    st_bf[:], src_ap, r1[:, nt_i:nt_i + 1], None,
                                    op0=mybir.AluOpType.mult,
                                )
                                nc.gpsimd.dma_start(
                                    ag_ins[nt_i][:, h * 512:(h + 1) * 512], st_bf[:])
                                sts.append((st_bf, nt_i, h))
                            nc.gpsimd.collective_compute(
                                "AllGather",
                                mybir.AluOpType.bypass,
                                ins=[ag_ins[nt_i].opt()],
                                outs=[ag_outs[nt_i].opt()],
                                replica_groups=[list(range(NC_CORES))],
                            )

                    for st_bf, nt_i, h in sts:
                        nc.gpsimd.dma_start(
                            scr1[h * 16:(h + 1) * 16, nt_i * 128:(nt_i + 1) * 128, :]
                            .transpose((1, 0, 2)),
                            st_bf[:].rearrange("p (c b) -> p c b", b=B),
                        )

                    for ch in range(NT):
                        nc.gpsimd.dma_start(
                            xgs[ch][C:2 * C, :].rearrange("c (n b) -> c n b", b=B),
                            scr1[:, ch * 128:(ch + 1) * 128, :],
                        )

                    # weight-slab generation — fills the AllGather bubble on PE
                    nc.gpsimd.dma_start(
                        xloc_sb[:].rearrange("p (t f) -> p t f", f=BC),
                        xt_loc[:, :].rearrange("(t p) f -> p t f", p=128),
                    )
                    with tc.tile_pool(name="ps_wt", bufs=4, space="PSUM") as ps_wt:
                        for o in range(O):
                            w_ps = ps_wt.tile([96, NL], F32, tag="wps")
                            nc.tensor.matmul(
                                w_ps[:],
                                wpo_sb[:, o * (CHEB_K * C):(o + 1) * (CHEB_K * C)],
                                etl_bf[:],
                                start=True, stop=True,
                            )
                            src_v = w_ps[:].rearrange("p (nh nl) -> p nh nl", nl=8)
                            if o % 2 == 0:
                                nc.vector.tensor_copy(wt_i8[:, :, o, :], src_v)
                            else:
                                nc.scalar.activation(wt_i8[:, :, o, :], src_v, AF.Copy)

                    # ---- hop2: k-outer, quarters consumed as AGs complete -
                    with tc.tile_pool(name="ps_h2", bufs=1, space="PSUM") as ps_h2:
                        acc2 = [
                            ps_h2.tile([128, 512], F32, tag=f"a2_{nt_i}_{h}",
                                       name=f"a2_{nt_i}_{h}")
                            for nt_i in range(NT) for h in range(2)
                        ]
                        for q in range(NT):
                            for r in range(NC_CORES):
                                rt = stream.tile([128, BC], BF16, tag="rhs", bufs=3)
                                nc.sync.dma_start(
                                    rt[:], ag_outs[q][r * 128:(r + 1) * 128, :])
                                k = r * NT + q
                                for nt_i in range(NT):
                                    lhs = pt[:, k * NL + nt_i * 128: k * NL + (nt_i + 1) * 128]
                                    for h in range(2):
                                        nc.tensor.matmul(
                                            acc2[nt_i * 2 + h][:],
                                            lhs,
                                            rt[:, h * 512:(h + 1) * 512],
                                            start=(q == 0 and r == 0),
                                            stop=(q == NT - 1 and r == NC_CORES - 1),
                                        )
                        for nt_i in range(NT):
                            for h in range(2):
                                a = acc2[nt_i * 2 + h]
                                st = stream.tile([128, 512], F32, tag="zst", bufs=2)
                                nc.scalar.activation(
                                    st[:], a[:], AF.Copy,
                                    scale=r2[:, nt_i:nt_i + 1],
                                )
                                st_bf = stream.tile([128, 512], BF16, tag="zstb")
                                nc.vector.tensor_tensor(
                                    st_bf[:], st[:],
                                    xloc_sb[:, nt_i * BC + h * 512: nt_i * BC + (h + 1) * 512],
                                    mybir.AluOpType.subtract,
                                )
                                nc.gpsimd.dma_start(
                                    scr2[h * 16:(h + 1) * 16, nt_i * 128:(nt_i + 1) * 128, :]
                                    .transpose((1, 0, 2)),
                                    st_bf[:].rearrange("p (c b) -> p c b", b=B),
                                )

                    engs = [nc.sync, nc.scalar, nc.gpsimd]
                    for ch in range(NT):
                        for h in range(2):
                            engs[(ch * 2 + h) % 3].dma_start(
                                xgs[ch][2 * C + 16 * h:2 * C + 16 * (h + 1), :]
                                .rearrange("c (n b) -> c n b", b=B),
                                scr2[16 * h:16 * (h + 1), ch * 128:(ch + 1) * 128, :],
                            )

                # ---- grouped per-node GEMM (col-tiled, 4 nodes concurrent) ----
                with tc.tile_pool(name="ps_g", bufs=8, space="PSUM") as ps_g:
                    for ch in range(NT):  # 128 nodes per chunk
                        n0 = ch * 128
                        xg_b = xgs[ch]
                        for q16 in range(4):  # 32 nodes per psum tile
                            g_ps = ps_g.tile([128, 512], F32, tag="gps")
                            for j in range(8):
                                for g in range(4):
                                    nl_i = q16 * 32 + j * 4 + g
                                    n_gl = n0 + nl_i
                                    nc.tensor.matmul(
                                        g_ps[32 * g:32 * (g + 1), j * O:(j + 1) * O],
                                        xg_b[:, nl_i * B:(nl_i + 1) * B],
                                        wt_g[:, n_gl // 8, n_gl % 8, :],
                                        start=True, stop=True,
                                        tile_position=(0, 32 * g),
                                    )
                            st = tstream.tile([128, 512], F32, tag="gst", bufs=2)
                            nc.vector.tensor_copy(st[:], g_ps[:])
                            dst4 = out_loc[:, n0 + q16 * 32: n0 + (q16 + 1) * 32, :] \
                                .rearrange("b (j g) o -> g b j o", g=4)
                            for g in range(4):
                                eng = (nc.sync, nc.gpsimd, nc.scalar, nc.sync)[g]
                                eng.dma_start(
                                    dst4[g],
                                    st[32 * g:32 * (g + 1), :].rearrange(
                                        "b (j o) -> b j o", o=O),
                                )
                xgp_cm.__exit__(None, None, None)
    return out_loc


_COMPILED = None


def _get_compiled():
    global _COMPILED
    if _COMPILED is None:
        nc = bacc.Bacc(
            "TRN2",
            target_bir_lowering=False,
            debug=False,
            num_devices=NC_CORES,
        )
        _build(nc)
        nc.compile()
        _COMPILED = nc
    return _COMPILED


def kernel(x, node_embeddings, laplacian_mx, weights_pool, bias_pool):
    x = np.asarray(x, dtype=np.float32)
    e = np.asarray(node_embeddings, dtype=np.float32)
    wp = np.asarray(weights_pool, dtype=np.float32)
    bp = np.asarray(bias_pool, dtype=np.float32)

    et = np.ascontiguousarray(e.T)                                  # [D, N]
    xt_h = np.ascontiguousarray(x.transpose(1, 2, 0).reshape(N, BC))  # [m, c*32+b]
    wpo = np.ascontiguousarray(wp.transpose(3, 0, 1, 2).reshape(O, D, CHEB_K * C)).astype(ml_dtypes.bfloat16)
    bias_h = (e @ bp).astype(np.float32)                            # [N, O]

    xt_b = xt_h.astype(ml_dtypes.bfloat16)
    in_maps = []
    for i in range(NC_CORES):
        sl = slice(i * NL, (i + 1) * NL)
        in_maps.append({
            "et": et,
            "et_loc": np.ascontiguousarray(et[:, sl]),
            "xt_bf": xt_b,
            "xtl_bf": np.ascontiguousarray(xt_b[sl]),
            "xt_loc": np.ascontiguousarray(xt_h[sl]),
            "wp_o": wpo,
            "bias_flat": np.ascontiguousarray(
                bias_h[sl].reshape(64, 8, O).transpose(0, 2, 1).reshape(1, NL * O).astype(ml_dtypes.bfloat16)
            ),
        })

    nc = _get_compiled()
    trace = bool(int(os.environ.get("KBENCH_TRACE", "0")))
    if trace:
        trace = _register_ntff_hook()
    res = run_bass_kernel_spmd(
        nc,
        in_maps,
        core_ids=list(range(NC_CORES)),
        trace=trace,
    )
    LAST_RESULTS["exec_time_ns"] = res.exec_time_ns
    LAST_RESULTS["trace"] = res.instructions_and_trace
    LAST_RESULTS["mean_exec_time_ns"] = res.mean_exec_time_ns

    out = np.empty((B, N, O), dtype=np.float32)
    for i in range(NC_CORES):
        out[:, i * NL:(i + 1) * NL, :] = res.results[i]["out_loc"]
    return out


# revision 32
# speedup vs baseline: 1.2140x; 1.0680x over previous
"""Trainium2 Bass kernel for nn_Embedded_GCN (gnn_message_passing).

Reference math (B=32, N=4096, C=32, O=64, D=16, K=3):
  A  = softmax(relu(E @ E.T), axis=1)              # [N, N] adaptive adjacency
  T0 = I, T1 = A, T2 = 2A@A - I                    # Chebyshev
  x_g[k]   = T_k @ x_b  for each batch             # [B, K, N, C]
  W[n]     = sum_d E[n,d] * Wp[d]                  # per-node weights [K,C,O]
  out[b,n] = sum_{k,i} x_g[b,n,k,i] W[n,k,i,:] + E[n]@bias_pool

Key restructurings:
  * T2 is never materialized: z1 = A@x, z2 = 2*A@z1 - x (avoids the N^3 matmul).
  * softmax(relu(s)) = max(1, exp(s)) / rowsum  (exp never overflows: logits <~ 50).
  * Scores are computed directly transposed: PT[m, n] = exp-scores with the
    contraction (m) on partitions, so the two propagation hops need no on-chip
    transpose of the 4096x4096 attention matrix.
  * Row sums via ones-matmul on the PE; 1/Z folded into the hop PSUM->SBUF
    epilogue as a per-partition activation scale.
  * Big matmuls run in float32r (FP22 truncation, full PE speed at N>=512).
  * Node-sharding across 8 cores (512 nodes each); z1 is AllGathered (2MB/rank).
  * Per-node grouped GEMM: x_g is permuted to [(k,c), (n,b)] via a DRAM
    round-trip (contiguous-ish chunks both directions), per-node weights are
    generated on the PE into a [97, (n,o)] slab (97th row = bias, matched by a
    ones-row in x_gT), and 512 small [97,32]x[97,64] bf16 matmuls finish it.
"""

import os

import numpy as np
import ml_dtypes

import concourse.bass as bass
import concourse.mybir as mybir
import concourse.tile as tile
from concourse import bacc
from concourse.bass_utils import run_bass_kernel_spmd

F32 = mybir.dt.float32
F32R = mybir.dt.float32r
BF16 = mybir.dt.bfloat16
AF = mybir.ActivationFunctionType

B, N, C, O, D, CHEB_K = 32, 4096, 32, 64, 16, 3
NC_CORES = 8
NL = N // NC_CORES          # 512 nodes per core
BC = B * C                  # 1024
MT = N // 128               # 32 contraction tiles
NT = NL // 128              # 4 local node tiles

LAST_RESULTS = {}


def _register_ntff_hook():
    """Inject antenv.axon_hooks (absent from the container's antenv stub) and
    register the ctypes NTFF-profile hook so trace=True works under axon."""
    import sys
    import types

    try:
        import antenv

        if "antenv.axon_hooks" not in sys.modules:
            mod = types.ModuleType("antenv.axon_hooks")
            mod._hook = None

            def set_axon_ntff_profile_hook(h):
                mod._hook = h

            def get_axon_ntff_profile_hook():
                return mod._hook

            mod.set_axon_ntff_profile_hook = set_axon_ntff_profile_hook
            mod.get_axon_ntff_profile_hook = get_axon_ntff_profile_hook
            sys.modules["antenv.axon_hooks"] = mod
            antenv.axon_hooks = mod

        hooks = sys.modules["antenv.axon_hooks"]
        if hooks.get_axon_ntff_profile_hook() is None:
            from trn_agent_boot.trn_boot import _ntff_profile_via_ctypes

            hook = _ntff_profile_via_ctypes("/opt/axon/libaxon_pjrt.so")
            if hook is not None:
                hooks.set_axon_ntff_profile_hook(hook)
        return True
    except Exception:
        return False


def _build(nc: bacc.Bacc):
    # ---- I/O -------------------------------------------------------------
    et = nc.dram_tensor("et", [D, N], F32, kind="ExternalInput")          # E^T
    et_loc = nc.dram_tensor("et_loc", [D, NL], F32, kind="ExternalInput")
    xt_bf = nc.dram_tensor("xt_bf", [N, BC], BF16, kind="ExternalInput")   # x[b,m,c] -> [m, c*32+b]
    xtl_bf = nc.dram_tensor("xtl_bf", [NL, BC], BF16, kind="ExternalInput")
    xt_loc = nc.dram_tensor("xt_loc", [NL, BC], F32, kind="ExternalInput")
    wp_o = nc.dram_tensor("wp_o", [O, D, CHEB_K * C], BF16, kind="ExternalInput")
    bias_flat = nc.dram_tensor("bias_flat", [1, NL * O], BF16, kind="ExternalInput")
    out_loc = nc.dram_tensor("out_loc", [B, NL, O], F32, kind="ExternalOutput")

    with tile.TileContext(nc) as tc:
        with tc.tile_pool(name="dram", bufs=1, space="DRAM") as dram, \
             tc.tile_pool(name="persist", bufs=1) as persist:

            ag_ins = [dram.tile([256, BC], BF16, tag=f"ag_in{q}", name=f"ag_in{q}")
                      for q in range(2)]
            ag_outs = [dram.tile([NC_CORES * 256, BC], BF16, tag=f"ag_out{q}",
                                 name=f"ag_out{q}", addr_space="Shared")
                       for q in range(2)]
            scr1 = dram.tile([C, NL, B], BF16, tag="scr1")   # z1 as [c, n, b]
            scr2 = dram.tile([C, NL, B], BF16, tag="scr2")   # z2 as [c, n, b]
            r_dram = dram.tile([2, NL], F32, tag="r_dram")

            # ---- small persistent SBUF ------------------------------------
            etl_sb = persist.tile([D, NL], F32R, tag="etl")
            r1 = persist.tile([128, NT], F32, tag="r1")          # 1/Z  per node col nt
            r2 = persist.tile([128, NT], F32, tag="r2")          # 2/Z

            etl_bf = persist.tile([D, NL], BF16, tag="etlbf")
            nc.sync.dma_start(etl_sb[:], et_loc[:, :].bitcast(F32R))
            nc.vector.tensor_copy(etl_bf[:], etl_sb[:].bitcast(F32))

            with tc.tile_pool(name="wtp", bufs=1) as wtp, \
                 tc.tile_pool(name="tstream", bufs=3) as tstream:
                # weight slab, n-major: [ki|bias, (n, o)]
                wt_bf = wtp.tile([97, NL * O], BF16, tag="wt")
                nc.gpsimd.dma_start(wt_bf[96:97, :], bias_flat[:, :])
                # interleaved: free idx = n_hi*512 + o*8 + n_lo, node n = n_hi*8 + n_lo
                wt_i8 = wt_bf[0:96, :].rearrange("p (nh o nl) -> p nh o nl", o=O, nl=8)
                wt_g = wt_bf[:].rearrange("p (nh o nl) -> p nh nl o", o=O, nl=8)

                xgp_cm = tc.tile_pool(name="xg", bufs=1)
                xgp = xgp_cm.__enter__()
                xgs = []
                for ch in range(NT):
                    n0 = ch * 128
                    xg_t = xgp.tile([97, 128 * B], BF16, tag=f"xg{ch}", name=f"xg{ch}")
                    nc.gpsimd.dma_start(
                        xg_t[0:C, :].rearrange("c (n b) -> c n b", b=B),
                        xtl_bf[n0:n0 + 128, :].rearrange("n (c b) -> c n b", b=B),
                    )
                    xgs.append(xg_t)

                wpo_sb = wtp.tile([D, O * CHEB_K * C], BF16, tag="wpo")
                nc.scalar.dma_start(
                    wpo_sb[:].rearrange("d (o k) -> d o k", k=CHEB_K * C),
                    wp_o[:, :, :].transpose((1, 0, 2)),
                )

                with tc.tile_pool(name="ptp", bufs=1) as ptp, \
                     tc.tile_pool(name="stream", bufs=3) as stream:
                    pt = ptp.tile([128, MT * NL], BF16, tag="pt")  # PT[m%128, mt*NL+n]
                    xloc_sb = ptp.tile([128, NT * BC], F32, tag="xloc")

                    # ---- phase B/C: transposed exp-scores + row sums ------
                    ones_f = persist.tile([128, 2], F32, tag="onesf")
                    nc.vector.memset(ones_f[:], 1.0)
                    with tc.tile_pool(name="etp", bufs=2) as etp, \
                         tc.tile_pool(name="ps_sc", bufs=3, space="PSUM") as ps_sc:
                        # interleaved accumulators: short critical path after last exp
                        accs = [ptp.tile([128, NL], F32, tag=f"accs{i}", name=f"accs{i}")
                                for i in range(2)]
                        et_c = None
                        for mt in range(MT):
                            if mt % 8 == 0:
                                et_c = etp.tile([D, 1024], F32R, tag="etc")
                                nc.sync.dma_start(
                                    et_c[:],
                                    et[:, mt * 128:(mt + 8) * 128].bitcast(F32R))
                            s_ps = ps_sc.tile([128, NL], F32, tag="s")
                            nc.tensor.matmul(
                                s_ps[:],
                                et_c[:, (mt % 8) * 128:(mt % 8 + 1) * 128],
                                etl_sb[:],
                                start=True, stop=True,
                            )
                            pslice = pt[:, mt * NL:(mt + 1) * NL]
                            nc.scalar.activation(pslice, s_ps[:], AF.Exp)
                            nc.vector.tensor_scalar_max(pslice, pslice, 1.0)
                            a = accs[mt % 2]
                            if mt < 2:
                                nc.vector.tensor_copy(a[:], pslice)
                            else:
                                nc.vector.tensor_tensor(
                                    a[:], a[:], pslice, mybir.AluOpType.add)
                        nc.vector.tensor_tensor(
                            accs[0][:], accs[0][:], accs[1][:], mybir.AluOpType.add)
                        acc_fin = accs[0]

                    for ch in range(NT):
                        nc.vector.memset(xgs[ch][96:97, :], 1.0)

                    # ---- hop1: k-outer, full-width rhs --------------------
                    with tc.tile_pool(name="ps_h1", bufs=1, space="PSUM") as ps_h1:
                        acc = [
                            ps_h1.tile([128, 512], F32, tag=f"acc{nt_i}_{h}",
                                       name=f"acc{nt_i}_{h}")
                            for nt_i in range(NT) for h in range(2)
                        ]
                        for k in range(MT):
                            rt = stream.tile([128, BC], BF16, tag="rhs", bufs=3)
                            nc.sync.dma_start(
                                rt[:], xt_bf[k * 128:(k + 1) * 128, :])
                            for nt_i in range(NT):
                                lhs = pt[:, k * NL + nt_i * 128: k * NL + (nt_i + 1) * 128]
                                for h in range(2):
                                    nc.tensor.matmul(
                                        acc[nt_i * 2 + h][:],
                                        lhs,
                                        rt[:, h * 512:(h + 1) * 512],
                                        start=(k == 0), stop=(k == MT - 1),
                                    )
                        # row-sums: save acc0 raw, reuse its bank for the zs matmuls
                        raw0 = stream.tile([128, 512], F32, tag="raw0", bufs=1)
                        nc.scalar.activation(raw0[:], acc[0][:], AF.Copy)
                        zs = ps_h1.tile([128, 8], F32, tag="acc0_0", name="zs")
                        for nt_i in range(NT):
                            nc.tensor.matmul(
                                zs[:, 2 * nt_i:2 * nt_i + 2],
                                acc_fin[:, nt_i * 128:(nt_i + 1) * 128],
                                ones_f[:],
                                start=True, stop=True,
                            )
                        for nt_i in range(NT):
                            nc.vector.reciprocal(
                                r1[:, nt_i:nt_i + 1], zs[:, 2 * nt_i:2 * nt_i + 1])
                            nc.vector.tensor_scalar_mul(
                                r2[:, nt_i:nt_i + 1], r1[:, nt_i:nt_i + 1], 2.0)

                        # epilogues: scale on DVE; per-nt AG triggered as soon
                        # as its slice is stored
                        sts = []
                        for nt_i in range(NT):
                            for h in range(2):
                                st_bf = stream.tile([128, 512], BF16, tag=f"zb{nt_i}_{h}",
                                                    name=f"zb{nt_i}_{h}", bufs=1)
                                src_ap = raw0[:] if (nt_i == 0 and h == 0) else acc[nt_i * 2 + h][:]
                                nc.vector.tensor_scalar(
                                    st_bf[:], src_ap, r1[:, nt_i:nt_i + 1], None,
                                    op0=mybir.AluOpType.mult,
                                )
                                nc.gpsimd.dma_start(
                                    ag_ins[nt_i // 2][(nt_i % 2) * 128:(nt_i % 2 + 1) * 128,
                                                      h * 512:(h + 1) * 512],
                                    st_bf[:])
                                sts.append((st_bf, nt_i, h))
                            if nt_i % 2 == 1:
                                nc.gpsimd.collective_compute(
                                    "AllGather",
                                    mybir.AluOpType.bypass,
                                    ins=[ag_ins[nt_i // 2].opt()],
                                    outs=[ag_outs[nt_i // 2].opt()],
                                    replica_groups=[list(range(NC_CORES))],
                                )

                    for st_bf, nt_i, h in sts:
                        nc.gpsimd.dma_start(
                            scr1[h * 16:(h + 1) * 16, nt_i * 128:(nt_i + 1) * 128, :]
                            .transpose((1, 0, 2)),
                            st_bf[:].rearrange("p (c b) -> p c b", b=B),
                        )

                    for ch in range(NT):
                        nc.gpsimd.dma_start(
                            xgs[ch][C:2 * C, :].rearrange("c (n b) -> c n b", b=B),
                            scr1[:, ch * 128:(ch + 1) * 128, :],
                        )

                    # weight-slab generation — fills the AllGather bubble on PE
                    nc.gpsimd.dma_start(
                        xloc_sb[:].rearrange("p (t f) -> p t f", f=BC),
                        xt_loc[:, :].rearrange("(t p) f -> p t f", p=128),
                    )
                    with tc.tile_pool(name="ps_wt", bufs=4, space="PSUM") as ps_wt:
                        for o in range(O):
                            w_ps = ps_wt.tile([96, NL], F32, tag="wps")
                            nc.tensor.matmul(
                                w_ps[:],
                                wpo_sb[:, o * (CHEB_K * C):(o + 1) * (CHEB_K * C)],
                                etl_bf[:],
                                start=True, stop=True,
                            )
                            src_v = w_ps[:].rearrange("p (nh nl) -> p nh nl", nl=8)
                            if o % 2 == 0:
                                nc.vector.tensor_copy(wt_i8[:, :, o, :], src_v)
                            else:
                                nc.scalar.activation(wt_i8[:, :, o, :], src_v, AF.Copy)

                    # ---- hop2: k-outer, quarters consumed as AGs complete -
                    with tc.tile_pool(name="ps_h2", bufs=1, space="PSUM") as ps_h2:
                        acc2 = [
                            ps_h2.tile([128, 512], F32, tag=f"a2_{nt_i}_{h}",
                                       name=f"a2_{nt_i}_{h}")
                            for nt_i in range(NT) for h in range(2)
                        ]
                        for q in range(2):
                            for rs in range(NC_CORES * 2):
                                r, s = rs // 2, rs % 2
                                rt = stream.tile([128, BC], BF16, tag="rhs", bufs=3)
                                nc.sync.dma_start(
                                    rt[:], ag_outs[q][(r * 2 + s) * 128:(r * 2 + s + 1) * 128, :])
                                k = r * NT + q * 2 + s
                                for nt_i in range(NT):
                                    lhs = pt[:, k * NL + nt_i * 128: k * NL + (nt_i + 1) * 128]
                                    for h in range(2):
                                        nc.tensor.matmul(
                                            acc2[nt_i * 2 + h][:],
                                            lhs,
                                            rt[:, h * 512:(h + 1) * 512],
                                            start=(q == 0 and rs == 0),
                                            stop=(q == 1 and rs == NC_CORES * 2 - 1),
                                        )
                        for nt_i in range(NT):
                            for h in range(2):
                                a = acc2[nt_i * 2 + h]
                                st = stream.tile([128, 512], F32, tag="zst", bufs=2)
                                nc.scalar.activation(
                                    st[:], a[:], AF.Copy,
                                    scale=r2[:, nt_i:nt_i + 1],
                                )
                                st_bf = stream.tile([128, 512], BF16, tag="zstb")
                                nc.vector.tensor_tensor(
                                    st_bf[:], st[:],
                                    xloc_sb[:, nt_i * BC + h * 512: nt_i * BC + (h + 1) * 512],
                                    mybir.AluOpType.subtract,
                                )
                                nc.gpsimd.dma_start(
                                    scr2[h * 16:(h + 1) * 16, nt_i * 128:(nt_i + 1) * 128, :]
                                    .transpose((1, 0, 2)),
                                    st_bf[:].rearrange("p (c b) -> p c b", b=B),
                                )

                    engs = [nc.sync, nc.scalar, nc.gpsimd]
                    for ch in range(NT):
                        for h in range(2):
                            engs[(ch * 2 + h) % 3].dma_start(
                                xgs[ch][2 * C + 16 * h:2 * C + 16 * (h + 1), :]
                                .rearrange("c (n b) -> c n b", b=B),
                                scr2[16 * h:16 * (h + 1), ch * 128:(ch + 1) * 128, :],
                            )

                # ---- grouped per-node GEMM (col-tiled, 4 nodes concurrent) ----
                with tc.tile_pool(name="ps_g", bufs=8, space="PSUM") as ps_g:
                    for ch in range(NT):  # 128 nodes per chunk
                        n0 = ch * 128
                        xg_b = xgs[ch]
                        for q16 in range(4):  # 32 nodes per psum tile
                            g_ps = ps_g.tile([128, 512], F32, tag="gps")
                            for j in range(8):
                                for g in range(4):
                                    nl_i = q16 * 32 + j * 4 + g
                                    n_gl = n0 + nl_i
                                    nc.tensor.matmul(
                                        g_ps[32 * g:32 * (g + 1), j * O:(j + 1) * O],
                                        xg_b[:, nl_i * B:(nl_i + 1) * B],
                                        wt_g[:, n_gl // 8, n_gl % 8, :],
                                        start=True, stop=True,
                                        tile_position=(0, 32 * g),
                                    )
                            st = tstream.tile([128, 512], F32, tag="gst", bufs=2)
                            nc.vector.tensor_copy(st[:], g_ps[:])
                            dst4 = out_loc[:, n0 + q16 * 32: n0 + (q16 + 1) * 32, :] \
                                .rearrange("b (j g) o -> g b j o", g=4)
                            for g in range(4):
                                eng = (nc.sync, nc.gpsimd, nc.scalar, nc.sync)[g]
                                eng.dma_start(
                                    dst4[g],
                                    st[32 * g:32 * (g + 1), :].rearrange(
                                        "b (j o) -> b j o", o=O),
                                )
                xgp_cm.__exit__(None, None, None)
    return out_loc


_COMPILED = None


def _get_compiled():
    global _COMPILED
    if _COMPILED is None:
        nc = bacc.Bacc(
            "TRN2",
            target_bir_lowering=False,
            debug=False,
            num_devices=NC_CORES,
        )
        _build(nc)
        nc.compile()
        _COMPILED = nc
    return _COMPILED


def kernel(x, node_embeddings, laplacian_mx, weights_pool, bias_pool):
    x = np.asarray(x, dtype=np.float32)
    e = np.asarray(node_embeddings, dtype=np.float32)
    wp = np.asarray(weights_pool, dtype=np.float32)
    bp = np.asarray(bias_pool, dtype=np.float32)

    et = np.ascontiguousarray(e.T)                                  # [D, N]
    xt_h = np.ascontiguousarray(x.transpose(1, 2, 0).reshape(N, BC))  # [m, c*32+b]
    wpo = np.ascontiguousarray(wp.transpose(3, 0, 1, 2).reshape(O, D, CHEB_K * C)).astype(ml_dtypes.bfloat16)
    bias_h = (e @ bp).astype(np.float32)                            # [N, O]

    xt_b = xt_h.astype(ml_dtypes.bfloat16)
    in_maps = []
    for i in range(NC_CORES):
        sl = slice(i * NL, (i + 1) * NL)
        in_maps.append({
            "et": et,
            "et_loc": np.ascontiguousarray(et[:, sl]),
            "xt_bf": xt_b,
            "xtl_bf": np.ascontiguousarray(xt_b[sl]),
            "xt_loc": np.ascontiguousarray(xt_h[sl]),
            "wp_o": wpo,
            "bias_flat": np.ascontiguousarray(
                bias_h[sl].reshape(64, 8, O).transpose(0, 2, 1).reshape(1, NL * O).astype(ml_dtypes.bfloat16)
            ),
        })

    nc = _get_compiled()
    trace = bool(int(os.environ.get("KBENCH_TRACE", "0")))
    if trace:
        trace = _register_ntff_hook()
    res = run_bass_kernel_spmd(
        nc,
        in_maps,
        core_ids=list(range(NC_CORES)),
        trace=trace,
    )
    LAST_RESULTS["exec_time_ns"] = res.exec_time_ns
    LAST_RESULTS["trace"] = res.instructions_and_trace
    LAST_RESULTS["mean_exec_time_ns"] = res.mean_exec_time_ns

    out = np.empty((B, N, O), dtype=np.float32)
    for i in range(NC_CORES):
        out[:, i * NL:(i + 1) * NL, :] = res.results[i]["out_loc"]
    return out
